# revision 82
# baseline (speedup 1.0000x reference)
"""Multi-head attention kernel for Trainium2 (8 NeuronCores, Bass/Tile).

Problem: B=2, S=2048, D=1024, H=16 heads (HD=64), causal mask, fp32.
Reference quirk: V is projected from the ALREADY-projected keys:
    k = keys @ Wk + bk ; v = k @ Wv + bv  =>  v = keys @ (Wk@Wv) + (bk@Wv + bv)

Sharding: core c handles batch b = c//4 and head-group g = c%4 (4 heads,
head-feature columns [256g, 256g+256)).  Each core:
  - projects q/k/v for its heads from its batch (contraction over full D),
  - computes full-sequence attention for its 4 heads,
  - produces a partial output  attn_g @ Wo[rows of g]  (row-parallel Wo).
Host sums the 4 partials per batch and adds bo.

v2 design (cost model: PE matmul = out_free x 0.417ns; engines in-order):
  - All activations/weights bf16 on device (halves DMA; same PE rate).
  - x inputs prepacked on host as [tci][kc][128][512] so one DMA per tci.
  - Scores S^T [128 ktok, 512 qtok] per k-chunk (pairs share one wide exp),
    diagonal chunks trimmed to their valid q-span [c0, 512).
  - PV flipped: out attn[128 qtok, 65] = pt_chunk.T @ [v_h | ones]; full
    128-partition utilization (old layout wasted half on replicated sums).
    Accumulated per (head, qsub) over k-chunks in a [128, 260] psum tile.
  - Normalize on DVE (reciprocal of col 64+65j, scalar-mul -> bf16 attn).
  - attn^T via DMA-transpose (XBAR) [128,128] tiles (zero engine time);
    the final block uses a PE transpose to skip the DMA round-trip.
  - Output projection from attnT; per-group weaving of projections/oproj
    keeps the PE fed through the ACT-paced attention phases (one PV group
    deferred so the in-order PE queue never waits on a fresh exp).
  - start=True on a matmul marks its whole 2KB psum bank pending-zero, so
    interleaved accumulation regions sharing a bank get exactly one start.
  - out DMA'd as bf16, host sums the 4 partials per batch in fp32.
"""
import sys
sys.path.insert(0, "/opt/trn_rl_repo")

import numpy as np
import ml_dtypes

import concourse.bacc as bacc
import concourse.mybir as mybir
import concourse.tile as tile
from concourse.bass_utils import run_bass_kernel_spmd

F32 = mybir.dt.float32
BF16 = mybir.dt.bfloat16
AF = mybir.ActivationFunctionType

B, S, D, H, HD = 2, 2048, 1024, 16, 64
NCORES = 8
HPC = 4            # heads per core
HF = HPC * HD      # 256 head-features per core
NKC = S // 128     # 16 k-chunks of 128 tokens
NQB = S // 512     # 4 q-blocks of 512 tokens
NDC = D // 128     # 8 contraction chunks for projections
SCALE = 1.0 / np.sqrt(HD)
VW = HD + 1        # v columns per head in vaug (64 v + 1 ones)
VROW = HPC * VW    # 260 vaug cols per k-chunk


def _classify_mask(mask):
    """Per (qblock 512, kchunk 128) x (qsub 128) classification of mask^T.

    Returns (plan, mask_tiles):
      plan[qb][kc] = None (fully masked -> skip) or (subs, c0, c1) where
        subs[j] in {('Z',), ('F',), ('M', idx)} and [c0, c1) is the valid
        q-span (first..last non-Z sub).
      mask_tiles: list of distinct [128,128] 0/1 tiles (transposed: [kt, qt]).
    """
    maskT = np.ascontiguousarray(mask.T)
    tiles = {}
    tiles_list = []
    plan = []
    for qb in range(NQB):
        row = []
        for kc in range(NKC):
            subT = maskT[kc * 128:(kc + 1) * 128, qb * 512:(qb + 1) * 512]
            subs = []
            for j in range(4):
                blk = subT[:, j * 128:(j + 1) * 128]
                if not blk.any():
                    subs.append(('Z',))
                elif blk.all():
                    subs.append(('F',))
                else:
                    key = blk.tobytes()
                    if key not in tiles:
                        tiles[key] = len(tiles_list)
                        tiles_list.append(blk)
                    subs.append(('M', tiles[key]))
            if all(s[0] == 'Z' for s in subs):
                row.append(None)
            else:
                nz = [j for j, s in enumerate(subs) if s[0] != 'Z']
                row.append((subs, nz[0] * 128, (nz[-1] + 1) * 128))
        plan.append(row)
    return plan, tiles_list


def _build_nc(plan, nmt, has_vbias, exp_group=2):
    nc = bacc.Bacc("TRN2", target_bir_lowering=False, debug=False)

    xq_d = nc.dram_tensor("xq", [NQB, NDC, 128, 512], BF16,
                          kind="ExternalInput").ap()
    xk_d = nc.dram_tensor("xk", [NQB, NDC, 128, 512], BF16,
                          kind="ExternalInput").ap()
    wq_d = nc.dram_tensor("wq", [NDC, 128, HF], BF16, kind="ExternalInput").ap()
    wk_d = nc.dram_tensor("wk", [NDC, 128, HF], BF16, kind="ExternalInput").ap()
    wkv_d = nc.dram_tensor("wkv", [NDC, 128, HF], BF16,
                           kind="ExternalInput").ap()
    wo_d = nc.dram_tensor("wo", [2, 128, D], BF16, kind="ExternalInput").ap()
    bqk_d = nc.dram_tensor("bqk", [128, 4], F32, kind="ExternalInput").ap()
    bkv_d = nc.dram_tensor("bkv", [1, HF], F32, kind="ExternalInput").ap()
    mt_d = nc.dram_tensor("mtiles", [max(nmt, 1), 128, 128], BF16,
                          kind="ExternalInput").ap()
    id_d = nc.dram_tensor("ident", [128, 128], BF16, kind="ExternalInput").ap()
    out_d = nc.dram_tensor("out", [S, D], BF16, kind="ExternalOutput").ap()

    with tile.TileContext(nc, pool_alloc_mode="stack") as tc:
        with tc.tile_pool(name="wpool", bufs=1) as wpool, \
             tc.tile_pool(name="big", bufs=1) as big, \
             tc.tile_pool(name="xpool", bufs=8) as xpool, \
             tc.tile_pool(name="ptpool", bufs=16) as ptpool, \
             tc.tile_pool(name="anpool", bufs=6) as anpool, \
             tc.tile_pool(name="rcpool", bufs=8) as rcpool, \
             tc.tile_pool(name="obpool", bufs=10) as obpool, \
             tc.tile_pool(name="stps", bufs=2, space="PSUM") as stps, \
             tc.tile_pool(name="atps", bufs=2, space="PSUM") as atps, \
             tc.tile_pool(name="pjps", bufs=2, space="PSUM") as pjps:

            # ---------------- weights / constants ----------------
            wq_sb = wpool.tile([128, NDC * HF], BF16, tag="wq")
            wk_sb = wpool.tile([128, NDC * HF], BF16, tag="wk")
            wkv_sb = wpool.tile([128, NDC * HF], BF16, tag="wkv")
            wo_sb = wpool.tile([128, 2 * D], BF16, tag="wo")
            bqk_sb = wpool.tile([128, 4], F32, tag="bqk")
            bq_sb = bqk_sb[:, 0:2]
            bk_sb = bqk_sb[:, 2:4]
            mt_sb = None
            if nmt > 0:
                mt_sb = wpool.tile([128, nmt * 128], BF16, tag="mt")
            id_sb = wpool.tile([128, 128], BF16, tag="id")
            if has_vbias:
                ones_sb = wpool.tile([1, 128], BF16, tag="ones")
                bkv_sb = wpool.tile([1, HF], F32, tag="bkv")
                bkv_bf = wpool.tile([1, HF], BF16, tag="bkvb")

            # persistent activations
            qT_sb = big.tile([128, 2 * S], BF16, tag="qT")
            kT_sb = big.tile([128, 2 * S], BF16, tag="kT")
            attnT_sb = big.tile([128, 2 * S], BF16, tag="attnT")
            vaug_sb = big.tile([128, NKC * VROW], BF16, tag="vaug")

            # ones columns of vaug: col 64 of every 65-col head group
            vaug4 = vaug_sb[:].rearrange("p (k h c) -> p k h c", h=HPC, c=VW)
            nc.gpsimd.memset(vaug4[:, :, :, HD:VW], 1.0)

            # ---------------- DMA emitters ----------------
            def emit_dma_half(dst_sb, src_d, half, nk=NDC // 2):
                # src [NDC, 128, HF] -> dst sbuf [128, NDC*HF], nk kc per call
                k0 = half * nk
                dst = dst_sb[:, k0 * HF:(k0 + nk) * HF]
                nc.sync.dma_start(
                    dst.rearrange("p (k c) -> p k c", k=nk),
                    src_d[k0:k0 + nk].rearrange("k p c -> p k c"))

            def emit_dma_quarter(dst_sb, src_d, quarter):
                emit_dma_half(dst_sb, src_d, quarter, nk=NDC // 4)

            def emit_x_dma(x_t, src_d, tci, half=None, quarter=None,
                           eng=None):
                eng = eng or nc.sync
                if half is None and quarter is None:
                    eng.dma_start(
                        x_t[:].rearrange("p (k c) -> p k c", k=NDC),
                        src_d[tci].rearrange("k p c -> p k c"))
                else:
                    nk = NDC // 2 if quarter is None else NDC // 4
                    k0 = (half if quarter is None else quarter) * nk
                    eng.dma_start(
                        x_t[:, k0 * 512:(k0 + nk) * 512].rearrange(
                            "p (k c) -> p k c", k=nk),
                        src_d[tci, k0:k0 + nk].rearrange("k p c -> p k c"))

            # ---------------- projection slice generators ----------------
            # Each generator yields small emit-units (~0.4-0.9us of PE work)
            # so they can be woven between attention score-groups.
            def gen_qproj(tci, x_t, step=1):
                PHASE_LOG.append((f"qproj{tci}", nc.get_next_instruction_name()))
                psq = [pjps.tile([128, 512], F32, tag="pj",
                                 name=f"psq{tci}_{hb}") for hb in range(2)]
                for kc0 in range(0, NDC, step):
                    def unit(kc0=kc0):
                        for kc in range(kc0, kc0 + step):
                            for hb in range(2):
                                nc.tensor.matmul(
                                    psq[hb][:],
                                    wq_sb[:, kc * HF + hb * 128:
                                          kc * HF + (hb + 1) * 128],
                                    x_t[:, kc * 512:(kc + 1) * 512],
                                    start=(kc == 0), stop=(kc == NDC - 1))
                    yield unit
                def cast():
                    for hb in range(2):
                        nc.vector.tensor_scalar_add(
                            qT_sb[:, hb * S + tci * 512:
                                  hb * S + (tci + 1) * 512],
                            psq[hb][:], bq_sb[:, hb:hb + 1])
                yield cast

            def gen_kproj(tci, x_t, step=1):
                PHASE_LOG.append((f"kproj{tci}", nc.get_next_instruction_name()))
                psk = [pjps.tile([128, 512], F32, tag="pj",
                                 name=f"psk{tci}_{hb}") for hb in range(2)]
                for kc0 in range(0, NDC, step):
                    def unit(kc0=kc0):
                        for kc in range(kc0, kc0 + step):
                            for hb in range(2):
                                nc.tensor.matmul(
                                    psk[hb][:],
                                    wk_sb[:, kc * HF + hb * 128:
                                          kc * HF + (hb + 1) * 128],
                                    x_t[:, kc * 512:(kc + 1) * 512],
                                    start=(kc == 0), stop=(kc == NDC - 1))
                    yield unit
                def cast():
                    for hb in range(2):
                        nc.vector.tensor_scalar_add(
                            kT_sb[:, hb * S + tci * 512:
                                  hb * S + (tci + 1) * 512],
                            psk[hb][:], bk_sb[:, hb:hb + 1])
                yield cast

            def gen_vproj(tci, x_t, split=2):
                for ts in range(4):
                    psv = pjps.tile([128, 512], F32, tag="pj",
                                    name=f"psv{tci}_{ts}")
                    for piece in range(split):
                        def unit(ts=ts, piece=piece, psv=psv):
                            kcs_ = range(piece * NDC // split,
                                         (piece + 1) * NDC // split)
                            for kc in kcs_:
                                nc.tensor.matmul(
                                    psv[:, :HF],
                                    x_t[:, kc * 512 + ts * 128:
                                        kc * 512 + (ts + 1) * 128],
                                    wkv_sb[:, kc * HF:(kc + 1) * HF],
                                    start=(kc == 0),
                                    stop=(kc == NDC - 1 and not has_vbias))
                            if piece != split - 1:
                                return
                            if has_vbias:
                                nc.tensor.matmul(psv[:, :HF], ones_sb[:],
                                                 bkv_bf[:], start=False,
                                                 stop=True)
                            kci = tci * 4 + ts
                            dst = vaug_sb[:, kci * VROW:(kci + 1) * VROW]
                            nc.vector.tensor_copy(
                                dst.rearrange("p (h c) -> p h c", c=VW)
                                   [:, :, 0:HD],
                                psv[:, :HF].rearrange("p (h c) -> p h c",
                                                      c=HD))
                        yield unit

            def gen_vproj0_paired(x_t):
                # startup-only: v accumulates in kc-half pieces so it can
                # start as soon as the first xk/wkv halves land; two ts share
                # one atps bank (idle before attention) via the one-start
                # pending-zero trick, keeping pjps free for kproj.
                for piece in range(2):
                    for tsp in range(2):
                        def unit(piece=piece, tsp=tsp):
                            psv = vps0[tsp]
                            for ts in (2 * tsp, 2 * tsp + 1):
                                col = (ts % 2) * HF
                                for kc in range(piece * NDC // 2,
                                                (piece + 1) * NDC // 2):
                                    nc.tensor.matmul(
                                        psv[:, col:col + HF],
                                        x_t[:, kc * 512 + ts * 128:
                                            kc * 512 + (ts + 1) * 128],
                                        wkv_sb[:, kc * HF:(kc + 1) * HF],
                                        start=(ts % 2 == 0 and kc == 0),
                                        stop=(kc == NDC - 1
                                              and not has_vbias),
                                        skip_group_check=True)
                                if piece != 1:
                                    continue
                                if has_vbias:
                                    nc.tensor.matmul(
                                        psv[:, col:col + HF], ones_sb[:],
                                        bkv_bf[:], start=False, stop=True,
                                        skip_group_check=True)
                                kci = ts
                                dst = vaug_sb[:, kci * VROW:(kci + 1) * VROW]
                                nc.vector.tensor_copy(
                                    dst.rearrange("p (h c) -> p h c", c=VW)
                                       [:, :, 0:HD],
                                    psv[:, col:col + HF].rearrange(
                                        "p (h c) -> p h c", c=HD))
                        yield unit

            def gen_oproj(tok0, tail=False):
                PHASE_LOG.append((f"oproj{tok0}", nc.get_next_instruction_name()))
                obuf = obpool.tile([128, D], BF16, tag="ob",
                                   name=f"ob{tok0}")
                wide = [None]
                for of in range(2):
                    def unit(of=of):
                        ci = (tok0 // 128) % 4
                        if tail and ci == 2:
                            # post-attention all psum pools are free: rotate
                            # tail chunks over pjps/atps/stps so no chunk
                            # waits on another's psum->sbuf copy
                            if wide[0] is None:
                                wide[0] = stps.tile(
                                    [128, 512 * exp_group], F32, tag="st",
                                    name=f"pow{tok0}")
                            ops = wide[0][:, of * 512:(of + 1) * 512]
                        elif tail and ci == 1:
                            ops = atps.tile([128, 512], F32, tag="at",
                                            name=f"po{tok0}_{of}")
                        else:
                            ops = pjps.tile([128, 512], F32, tag="pj",
                                            name=f"po{tok0}_{of}")
                        for hb2 in range(2):
                            nc.tensor.matmul(
                                ops[:],
                                attnT_sb[:, hb2 * S + tok0:
                                         hb2 * S + tok0 + 128],
                                wo_sb[:, hb2 * D + of * 512:
                                      hb2 * D + (of + 1) * 512],
                                start=(hb2 == 0), stop=(hb2 == 1))
                        if tail and (of + tok0 // 128) % 2 == 0:
                            # tail: split the psum->sbuf copies across ACT
                            # and DVE (alternating per chunk) so neither
                            # engine's in-order queue paces the drain
                            nc.scalar.activation(
                                obuf[:, of * 512:(of + 1) * 512], ops[:],
                                AF.Identity, bias=0.0, scale=1.0)
                        else:
                            nc.vector.tensor_copy(
                                obuf[:, of * 512:(of + 1) * 512], ops[:])
                        if tail and tok0 == S - 128:
                            # last chunks: per-half DMAs shorten the final
                            # transfer on the critical path
                            nc.sync.dma_start(
                                out_d[tok0:tok0 + 128,
                                      of * 512:(of + 1) * 512],
                                obuf[:, of * 512:(of + 1) * 512])
                        elif of == 1:
                            nc.sync.dma_start(out_d[tok0:tok0 + 128, :],
                                              obuf[:])
                    yield unit

            def gen_load_wo():
                def unit():
                    nc.sync.dma_start(
                        wo_sb[:].rearrange("p (b c) -> p b c", b=2),
                        wo_d.rearrange("b p c -> p b c"))
                    nc.sync.dma_start(id_sb[:], id_d)
                yield unit

            def gen_load_masks():
                def unit():
                    for i in range(nmt):
                        nc.sync.dma_start(mt_sb[:, i * 128:(i + 1) * 128],
                                          mt_d[i])
                yield unit

            # ---------------- attention ----------------
            # Cross-group/head/qb deferral state: "pv" is the previous
            # group's PV emission, "norm" the completed heads' normalize+
            # transpose emissions.  Flushed after the next group's exp so
            # the in-order PE queue never head-blocks on a fresh exp.
            pend = {"pv": None, "norm": []}

            def flush_pending():
                if pend["pv"] is not None:
                    pend["pv"]()
                    pend["pv"] = None
                while pend["norm"]:
                    pend["norm"].pop(0)()

            def emit_attention(qb, extras):
                PHASE_LOG.append((f"att{qb}", nc.get_next_instruction_name()))
                q0 = qb * 512
                kcs = [kc for kc in range(NKC) if plan[qb][kc] is not None]
                # Group chunks in pairs whose exp span has no unwritten gap:
                # a pair (A, B) spans [c0_A, 512) u [512 + c0_B, ...), which
                # is contiguous iff c0_B == 0.  Pair trimmed (c0>0) chunks
                # with zero-c0 chunks (late ones, to respect weave deps);
                # leftover trimmed chunks become singleton groups.
                gaps_l = [kc for kc in kcs if plan[qb][kc][1] > 0]
                zeros_l = [kc for kc in kcs if plan[qb][kc][1] == 0]
                n_m = min(len(gaps_l), len(zeros_l))
                mixed = [[gaps_l[len(gaps_l) - n_m + i],
                          zeros_l[len(zeros_l) - n_m + i]]
                         for i in range(n_m)]
                rest_z = zeros_l[:len(zeros_l) - n_m]
                rest_g = gaps_l[:len(gaps_l) - n_m]
                groups = [rest_z[i:i + exp_group]
                          for i in range(0, len(rest_z), exp_group)]
                groups += mixed
                groups += [[kc] for kc in rest_g]
                # first valid kc per qsub + last EMITTED kc per qsub (for
                # PV psum stop flags), following the group emission order
                order = [kc for grp in groups for kc in grp]
                first_kc = {}
                last_kc = {}
                for kc in order:
                    subs = plan[qb][kc][0]
                    for j in range(4):
                        if subs[j][0] != 'Z':
                            first_kc.setdefault(j, kc)
                            last_kc[j] = kc
                front, spread = extras
                n_groups = len(groups) * HPC
                n_spread = len(spread)
                g_idx = 0
                popped = 0
                spread_start = [None]
                for h in range(HPC):
                    hb, hr = h // 2, (h % 2) * 64
                    # Bank-sized (2KB) tile: matmul start=True zeroes the
                    # whole 2KB psum bank, so only the FIRST PV matmul of
                    # this (head, qb) may carry start=True; later qsubs'
                    # first writes land on still-pending bytes and
                    # initialize correctly.
                    attn_ps = atps.tile([128, 512], F32, tag="at",
                                        name=f"at{qb}_{h}")
                    state = {"started": False}
                    for grp in groups:
                        # front extras carry dependencies for upcoming phases:
                        # pop one per group ASAP.  spread extras (oproj) are
                        # dependency-free; schedule them uniformly over the
                        # remaining groups so ACT-paced stretches keep the PE
                        # fed to the very end of the phase.
                        g_idx += 1
                        if front:
                            front.pop(0)()
                        elif spread:
                            if spread_start[0] is None:
                                spread_start[0] = g_idx
                            done = g_idx - spread_start[0] + 1
                            span = max(n_groups - spread_start[0] + 1, 1)
                            target = done * n_spread // span
                            while spread and popped < target:
                                spread.pop(0)()
                                popped += 1
                        g = len(grp)
                        st = stps.tile([128, 512 * exp_group], F32, tag="st")
                        pt = ptpool.tile([128, 512 * exp_group], BF16,
                                         tag="pt")
                        runs = []
                        for i, kc in enumerate(grp):
                            c0, c1 = plan[qb][kc][1], plan[qb][kc][2]
                            nc.tensor.matmul(
                                st[:, i * 512 + c0:i * 512 + c1],
                                kT_sb[hr:hr + 64,
                                      hb * S + kc * 128:
                                      hb * S + (kc + 1) * 128],
                                qT_sb[hr:hr + 64,
                                      hb * S + q0 + c0:hb * S + q0 + c1],
                                start=True, stop=True)
                            lo, hi = i * 512 + c0, i * 512 + c1
                            if runs and runs[-1][1] == lo:
                                runs[-1][1] = hi
                            else:
                                runs.append([lo, hi])
                        # exp only over written psum (one op per run)
                        for e0, e1 in runs:
                            nc.scalar.activation(pt[:, e0:e1], st[:, e0:e1],
                                                 AF.Exp, bias=0.0,
                                                 scale=float(SCALE))
                        for i, kc in enumerate(grp):
                            subs = plan[qb][kc][0]
                            base = i * 512
                            for j, sub in enumerate(subs):
                                if sub[0] == 'M':
                                    idx = sub[1]
                                    nc.vector.tensor_mul(
                                        pt[:, base + j * 128:
                                           base + (j + 1) * 128],
                                        pt[:, base + j * 128:
                                           base + (j + 1) * 128],
                                        mt_sb[:, idx * 128:(idx + 1) * 128])
                        # PV for this group is deferred one group so the PE
                        # queue head never waits on this group's exp.
                        flush_pending()
                        def pv(qb=qb, h=h, grp=grp, pt=pt,
                               attn_ps=attn_ps, state=state,
                               last_kc=last_kc):
                            for i, kc in enumerate(grp):
                                subs = plan[qb][kc][0]
                                base = i * 512
                                for j, sub in enumerate(subs):
                                    if sub[0] == 'Z':
                                        continue
                                    nc.tensor.matmul(
                                        attn_ps[:, j * VW:(j + 1) * VW],
                                        pt[:, base + j * 128:
                                           base + (j + 1) * 128],
                                        vaug_sb[:, kc * VROW + h * VW:
                                                kc * VROW + (h + 1) * VW],
                                        start=not state["started"],
                                        stop=(kc == last_kc[j]),
                                        skip_group_check=True)
                                    state["started"] = True
                        pend["pv"] = pv

                    def norm(qb=qb, h=h, hb=hb, q0=q0, attn_ps=attn_ps,
                             first_kc=first_kc, attn_qb=attn_qb):
                        rcp = rcpool.tile([128, 4], F32, tag="rc",
                                          name=f"rc{qb}_{h}")
                        nc.vector.reciprocal(
                            rcp[:],
                            attn_ps[:, :HPC * VW]
                            .rearrange("p (j c) -> p j c", c=VW)
                            [:, :, HD:HD + 1])
                        if all(j in first_kc for j in range(4)):
                            # all qsubs valid: one broadcast mul for the head
                            nc.vector.tensor_mul(
                                attn_qb[:, hb * 512:(hb + 1) * 512]
                                .rearrange("p (j two c) -> p j two c",
                                           j=4, two=2, c=64)[:, :, h % 2, :],
                                attn_ps[:, :HPC * VW]
                                .rearrange("p (j c) -> p j c", c=VW)
                                [:, :, 0:HD],
                                rcp[:].rearrange("p (j c) -> p j c", c=1)
                                .broadcast_to([128, 4, HD]))
                        else:
                            for j in range(4):
                                if j not in first_kc:
                                    continue
                                nc.vector.tensor_scalar_mul(
                                    attn_qb[:, hb * 512 + j * 128
                                            + (h % 2) * 64:
                                            hb * 512 + j * 128
                                            + (h % 2) * 64 + 64],
                                    attn_ps[:, j * VW:j * VW + HD],
                                    rcp[:, j:j + 1])
                        if h % 2 == 1:  # hb complete -> transpose
                            if qb == NQB - 1 and hb == 1:
                                # final block: PE transpose (scores are done,
                                # st psum is free) avoids the ~2us DMA
                                # round-trip on the tail critical path
                                tps = stps.tile([128, 512 * exp_group], BF16,
                                                tag="st", name="tps")
                                for j in range(4):
                                    nc.tensor.transpose(
                                        tps[:, j * 128:(j + 1) * 128],
                                        attn_qb[:, hb * 512 + j * 128:
                                                hb * 512 + (j + 1) * 128],
                                        id_sb[:])
                                nc.vector.tensor_copy(
                                    attnT_sb[:, hb * S + q0:
                                             hb * S + q0 + 512],
                                    tps[:, 0:512])
                            else:
                                for j in range(4):
                                    nc.sync.dma_start_transpose(
                                        attnT_sb[:, hb * S + q0 + j * 128:
                                                 hb * S + q0 + (j + 1) * 128],
                                        attn_qb[:, hb * 512 + j * 128:
                                                hb * 512 + (j + 1) * 128])
                    pend["norm"].append(norm)

            # ---------------- emission ----------------
            x_tiles = {}
            # qproj(0): weave weight-quarter DMAs with x-quarter DMAs so the
            # first matmul can start ~2.8us in.  Biases deferred (only the
            # casts need them).
            xq0 = xpool.tile([128, NDC * 512], BF16, tag="x", name="xq0")
            x_tiles[("q", 0)] = xq0
            for quarter in range(2):
                emit_dma_quarter(wq_sb, wq_d, quarter)
                emit_x_dma(xq0, xq_d, 0, quarter=quarter, eng=nc.gpsimd)
            emit_dma_half(wq_sb, wq_d, 1)
            emit_x_dma(xq0, xq_d, 0, half=1, eng=nc.gpsimd)
            nc.sync.dma_start(bqk_sb[:], bqk_d)
            for u in gen_qproj(0, xq0, step=2):
                u()
            xk0 = xpool.tile([128, NDC * 512], BF16, tag="x", name="xk0")
            x_tiles[("k", 0)] = xk0
            for half in range(2):
                emit_dma_half(wk_sb, wk_d, half)
                emit_x_dma(xk0, xk_d, 0, half=half, eng=nc.gpsimd)
                emit_dma_half(wkv_sb, wkv_d, half)
            if has_vbias:
                nc.gpsimd.memset(ones_sb[:], 1.0)
                nc.sync.dma_start(bkv_sb[:], bkv_d)
                nc.vector.tensor_copy(bkv_bf[:], bkv_sb[:])
            vps0 = [atps.tile([128, 512], F32, tag="at", name=f"vps0_{i}")
                    for i in range(2)]
            ku = list(gen_kproj(0, xk0, step=2))
            vu = list(gen_vproj0_paired(xk0))
            for u in (ku[0], ku[1], vu[0], vu[1],
                      ku[2], ku[3], ku[4], vu[2], vu[3]):
                u()
            if nmt > 0:
                for u in gen_load_masks():
                    u()
            # prefetch x for tci 1; tci 2,3 queued later
            for tci in (1, 2, 3):
                xq = xpool.tile([128, NDC * 512], BF16, tag="x",
                                name=f"xq{tci}")
                xk = xpool.tile([128, NDC * 512], BF16, tag="x",
                                name=f"xk{tci}")
                x_tiles[("q", tci)] = xq
                x_tiles[("k", tci)] = xk
            emit_x_dma(x_tiles[("q", 1)], xq_d, 1)
            emit_x_dma(x_tiles[("k", 1)], xk_d, 1)

            attn_qb = None

            def slices(*gens):
                out = []
                for g in gens:
                    out.extend(g)
                return out

            def defer_dma(*calls):
                def unit():
                    for c in calls:
                        c()
                return [unit]

            # Extras per attention phase, sized to each phase's PE deficit
            # (ACT-paced groups leave ~400ns/group of PE idle unless filled).
            # Hard deps: qT/kT of tci must exist before att(tci) scores that
            # read them; vaug of tci before att PV reaches those k-chunks.
            ex0 = (slices(
                gen_qproj(1, x_tiles[("q", 1)], step=2),
                defer_dma(lambda: emit_x_dma(x_tiles[("q", 2)], xq_d, 2),
                          lambda: emit_x_dma(x_tiles[("k", 2)], xk_d, 2)),
                gen_kproj(1, x_tiles[("k", 1)], step=2),
                gen_vproj(1, x_tiles[("k", 1)], split=1),
            ), [])
            ex1 = (slices(
                gen_load_wo(),
                gen_qproj(2, x_tiles[("q", 2)], step=2),
                defer_dma(lambda: emit_x_dma(x_tiles[("q", 3)], xq_d, 3),
                          lambda: emit_x_dma(x_tiles[("k", 3)], xk_d, 3)),
                gen_kproj(2, x_tiles[("k", 2)], step=2),
            ), [])
            ex2 = (slices(
                gen_vproj(2, x_tiles[("k", 2)], split=1),
                gen_qproj(3, x_tiles[("q", 3)], step=2),
            ), slices(
                gen_oproj(0 * 512 + 0 * 128),
                gen_oproj(0 * 512 + 1 * 128),
            ))
            ex3 = (slices(
                gen_kproj(3, x_tiles[("k", 3)], step=2),
                gen_vproj(3, x_tiles[("k", 3)], split=1),
            ), slices(
                gen_oproj(0 * 512 + 2 * 128),
                gen_oproj(0 * 512 + 3 * 128),
                gen_oproj(1 * 512 + 0 * 128),
                gen_oproj(1 * 512 + 1 * 128),
                gen_oproj(1 * 512 + 2 * 128),
                gen_oproj(1 * 512 + 3 * 128),
                gen_oproj(2 * 512 + 0 * 128),
                gen_oproj(2 * 512 + 1 * 128),
                gen_oproj(2 * 512 + 2 * 128),
                gen_oproj(2 * 512 + 3 * 128),
            ))
            extras = [ex0, ex1, ex2, ex3]
            for qb in range(NQB):
                attn_qb = anpool.tile([128, 1024], BF16, tag="an",
                                      name=f"an{qb}")
                emit_attention(qb, extras[qb])
                for lst in extras[qb]:
                    while lst:
                        lst.pop(0)()
            flush_pending()
            for t in range(4):
                for u in gen_oproj(3 * 512 + t * 128, tail=True):
                    u()
    nc.compile()
    return nc


_CACHE = {}
PHASE_LOG = []
VARIANT = {"exp_group": 2}


def _get_nc(plan, nmt, has_vbias):
    key = (repr(plan), nmt, has_vbias, repr(sorted(VARIANT.items())))
    if key not in _CACHE:
        _CACHE[key] = _build_nc(plan, nmt, has_vbias, **VARIANT)
    return _CACHE[key]


def shard_inputs(queries, keys, mask, Wq, bq, Wk, bk, Wv, bv, Wo, bo):
    """Host-side prep: returns (in_maps, plan, nmt, has_vbias)."""
    Wkv = (Wk.astype(np.float64) @ Wv.astype(np.float64)).astype(np.float32)
    bkv = (bk.astype(np.float64) @ Wv.astype(np.float64)
           + bv.astype(np.float64)).astype(np.float32)
    has_vbias = bool(np.any(bkv != 0.0))

    plan, tiles_list = _classify_mask(np.asarray(mask))
    nmt = len(tiles_list)
    assert nmt <= 64, f"too many distinct mask tiles ({nmt})"
    if nmt > 0:
        mtiles = np.stack(tiles_list).astype(ml_dtypes.bfloat16)
    else:
        mtiles = np.zeros((1, 128, 128), dtype=ml_dtypes.bfloat16)

    def pack_x(x):
        # [S, D] -> [tci, kc, 128 d, 512 tok] bf16
        return np.ascontiguousarray(
            x.reshape(NQB, 512, NDC, 128).transpose(0, 2, 3, 1)
        ).astype(ml_dtypes.bfloat16)

    def pack_w(w, cols):
        return np.ascontiguousarray(
            w[:, cols].reshape(NDC, 128, HF)).astype(ml_dtypes.bfloat16)

    in_maps = []
    for c in range(NCORES):
        b, g = c // 4, c % 4
        cols = slice(HF * g, HF * (g + 1))
        in_maps.append({
            "xq": pack_x(queries[b]),
            "xk": pack_x(keys[b]),
            "wq": pack_w(Wq, cols),
            "wk": pack_w(Wk, cols),
            "wkv": pack_w(Wkv, cols),
            "wo": np.ascontiguousarray(
                Wo[cols, :].reshape(2, 128, D)).astype(ml_dtypes.bfloat16),
            "bqk": np.ascontiguousarray(np.concatenate(
                [bq[cols].reshape(2, 128).T, bk[cols].reshape(2, 128).T],
                axis=1)),
            "bkv": bkv[cols].reshape(1, HF).copy(),
            "mtiles": mtiles,
            "ident": np.eye(128, dtype=ml_dtypes.bfloat16),
        })
    return in_maps, plan, nmt, has_vbias


def combine_outputs(results, bo):
    out = np.empty((B, S, D), dtype=np.float32)
    for b in range(B):
        acc = results[4 * b]["out"].astype(np.float32)
        for g in range(1, 4):
            acc = acc + results[4 * b + g]["out"].astype(np.float32)
        out[b] = acc + bo[None, :]
    return out


def kernel(queries, keys, values, mask, Wq, bq, Wk, bk, Wv, bv, Wo, bo,
           _trace=False, _result_holder=None):
    queries = np.asarray(queries, dtype=np.float32)
    keys = np.asarray(keys, dtype=np.float32)
    mask = np.asarray(mask)
    in_maps, plan, nmt, has_vbias = shard_inputs(
        queries, keys, mask,
        np.asarray(Wq, np.float32), np.asarray(bq, np.float32),
        np.asarray(Wk, np.float32), np.asarray(bk, np.float32),
        np.asarray(Wv, np.float32), np.asarray(bv, np.float32),
        np.asarray(Wo, np.float32), np.asarray(bo, np.float32))
    nc = _get_nc(plan, nmt, has_vbias)
    res = run_bass_kernel_spmd(nc, in_maps, core_ids=list(range(NCORES)),
                               trace=_trace)
    if _result_holder is not None:
        _result_holder.append(res)
    return combine_outputs(res.results, np.asarray(bo, np.float32))


# revision 89
# speedup vs baseline: 1.0012x; 1.0012x over previous
"""Multi-head attention kernel for Trainium2 (8 NeuronCores, Bass/Tile).

Problem: B=2, S=2048, D=1024, H=16 heads (HD=64), causal mask, fp32.
Reference quirk: V is projected from the ALREADY-projected keys:
    k = keys @ Wk + bk ; v = k @ Wv + bv  =>  v = keys @ (Wk@Wv) + (bk@Wv + bv)

Sharding: core c handles batch b = c//4 and head-group g = c%4 (4 heads,
head-feature columns [256g, 256g+256)).  Each core:
  - projects q/k/v for its heads from its batch (contraction over full D),
  - computes full-sequence attention for its 4 heads,
  - produces a partial output  attn_g @ Wo[rows of g]  (row-parallel Wo).
Host sums the 4 partials per batch and adds bo.

v2 design (cost model: PE matmul = out_free x 0.417ns; engines in-order):
  - All activations/weights bf16 on device (halves DMA; same PE rate).
  - x inputs prepacked on host as [tci][kc][128][512] so one DMA per tci.
  - Scores S^T [128 ktok, 512 qtok] per k-chunk (pairs share one wide exp),
    diagonal chunks trimmed to their valid q-span [c0, 512).
  - PV flipped: out attn[128 qtok, 65] = pt_chunk.T @ [v_h | ones]; full
    128-partition utilization (old layout wasted half on replicated sums).
    Accumulated per (head, qsub) over k-chunks in a [128, 260] psum tile.
  - Normalize on DVE (reciprocal of col 64+65j, scalar-mul -> bf16 attn).
  - attn^T via DMA-transpose (XBAR) [128,128] tiles (zero engine time);
    the final block uses a PE transpose to skip the DMA round-trip.
  - Output projection from attnT; per-group weaving of projections/oproj
    keeps the PE fed through the ACT-paced attention phases (one PV group
    deferred so the in-order PE queue never waits on a fresh exp).
  - start=True on a matmul marks its whole 2KB psum bank pending-zero, so
    interleaved accumulation regions sharing a bank get exactly one start.
  - out DMA'd as bf16, host sums the 4 partials per batch in fp32.
"""
import sys
sys.path.insert(0, "/opt/trn_rl_repo")

import numpy as np
import ml_dtypes

import concourse.bacc as bacc
import concourse.mybir as mybir
import concourse.tile as tile
from concourse.bass_utils import run_bass_kernel_spmd

F32 = mybir.dt.float32
BF16 = mybir.dt.bfloat16
AF = mybir.ActivationFunctionType

B, S, D, H, HD = 2, 2048, 1024, 16, 64
NCORES = 8
HPC = 4            # heads per core
HF = HPC * HD      # 256 head-features per core
NKC = S // 128     # 16 k-chunks of 128 tokens
NQB = S // 512     # 4 q-blocks of 512 tokens
NDC = D // 128     # 8 contraction chunks for projections
SCALE = 1.0 / np.sqrt(HD)
VW = HD + 1        # v columns per head in vaug (64 v + 1 ones)
VROW = HPC * VW    # 260 vaug cols per k-chunk


def _classify_mask(mask):
    """Per (qblock 512, kchunk 128) x (qsub 128) classification of mask^T.

    Returns (plan, mask_tiles):
      plan[qb][kc] = None (fully masked -> skip) or (subs, c0, c1) where
        subs[j] in {('Z',), ('F',), ('M', idx)} and [c0, c1) is the valid
        q-span (first..last non-Z sub).
      mask_tiles: list of distinct [128,128] 0/1 tiles (transposed: [kt, qt]).
    """
    maskT = np.ascontiguousarray(mask.T)
    tiles = {}
    tiles_list = []
    plan = []
    for qb in range(NQB):
        row = []
        for kc in range(NKC):
            subT = maskT[kc * 128:(kc + 1) * 128, qb * 512:(qb + 1) * 512]
            subs = []
            for j in range(4):
                blk = subT[:, j * 128:(j + 1) * 128]
                if not blk.any():
                    subs.append(('Z',))
                elif blk.all():
                    subs.append(('F',))
                else:
                    key = blk.tobytes()
                    if key not in tiles:
                        tiles[key] = len(tiles_list)
                        tiles_list.append(blk)
                    subs.append(('M', tiles[key]))
            if all(s[0] == 'Z' for s in subs):
                row.append(None)
            else:
                nz = [j for j, s in enumerate(subs) if s[0] != 'Z']
                row.append((subs, nz[0] * 128, (nz[-1] + 1) * 128))
        plan.append(row)
    return plan, tiles_list


def _build_nc(plan, nmt, has_vbias, exp_group=2):
    nc = bacc.Bacc("TRN2", target_bir_lowering=False, debug=False)

    xq_d = nc.dram_tensor("xq", [NQB, NDC, 128, 512], BF16,
                          kind="ExternalInput").ap()
    xk_d = nc.dram_tensor("xk", [NQB, NDC, 128, 512], BF16,
                          kind="ExternalInput").ap()
    wq_d = nc.dram_tensor("wq", [NDC, 128, HF], BF16, kind="ExternalInput").ap()
    wk_d = nc.dram_tensor("wk", [NDC, 128, HF], BF16, kind="ExternalInput").ap()
    wkv_d = nc.dram_tensor("wkv", [NDC, 128, HF], BF16,
                           kind="ExternalInput").ap()
    wo_d = nc.dram_tensor("wo", [2, 128, D], BF16, kind="ExternalInput").ap()
    bqk_d = nc.dram_tensor("bqk", [128, 4], F32, kind="ExternalInput").ap()
    bkv_d = nc.dram_tensor("bkv", [1, HF], F32, kind="ExternalInput").ap()
    mt_d = nc.dram_tensor("mtiles", [max(nmt, 1), 128, 128], BF16,
                          kind="ExternalInput").ap()
    id_d = nc.dram_tensor("ident", [128, 128], BF16, kind="ExternalInput").ap()
    out_d = nc.dram_tensor("out", [S, D], BF16, kind="ExternalOutput").ap()

    with tile.TileContext(nc, pool_alloc_mode="stack") as tc:
        with tc.tile_pool(name="wpool", bufs=1) as wpool, \
             tc.tile_pool(name="big", bufs=1) as big, \
             tc.tile_pool(name="xpool", bufs=8) as xpool, \
             tc.tile_pool(name="ptpool", bufs=16) as ptpool, \
             tc.tile_pool(name="anpool", bufs=6) as anpool, \
             tc.tile_pool(name="rcpool", bufs=8) as rcpool, \
             tc.tile_pool(name="obpool", bufs=10) as obpool, \
             tc.tile_pool(name="stps", bufs=2, space="PSUM") as stps, \
             tc.tile_pool(name="atps", bufs=2, space="PSUM") as atps, \
             tc.tile_pool(name="pjps", bufs=2, space="PSUM") as pjps:

            # ---------------- weights / constants ----------------
            wq_sb = wpool.tile([128, NDC * HF], BF16, tag="wq")
            wk_sb = wpool.tile([128, NDC * HF], BF16, tag="wk")
            wkv_sb = wpool.tile([128, NDC * HF], BF16, tag="wkv")
            wo_sb = wpool.tile([128, 2 * D], BF16, tag="wo")
            bqk_sb = wpool.tile([128, 4], F32, tag="bqk")
            bq_sb = bqk_sb[:, 0:2]
            bk_sb = bqk_sb[:, 2:4]
            mt_sb = None
            if nmt > 0:
                mt_sb = wpool.tile([128, nmt * 128], BF16, tag="mt")
            id_sb = wpool.tile([128, 128], BF16, tag="id")
            if has_vbias:
                ones_sb = wpool.tile([1, 128], BF16, tag="ones")
                bkv_sb = wpool.tile([1, HF], F32, tag="bkv")
                bkv_bf = wpool.tile([1, HF], BF16, tag="bkvb")

            # persistent activations
            qT_sb = big.tile([128, 2 * S], BF16, tag="qT")
            kT_sb = big.tile([128, 2 * S], BF16, tag="kT")
            attnT_sb = big.tile([128, 2 * S], BF16, tag="attnT")
            vaug_sb = big.tile([128, NKC * VROW], BF16, tag="vaug")

            # ones columns of vaug: col 64 of every 65-col head group
            vaug4 = vaug_sb[:].rearrange("p (k h c) -> p k h c", h=HPC, c=VW)
            nc.gpsimd.memset(vaug4[:, :, :, HD:VW], 1.0)

            # ---------------- DMA emitters ----------------
            def emit_dma_half(dst_sb, src_d, half, nk=NDC // 2):
                # src [NDC, 128, HF] -> dst sbuf [128, NDC*HF], nk kc per call
                k0 = half * nk
                dst = dst_sb[:, k0 * HF:(k0 + nk) * HF]
                nc.sync.dma_start(
                    dst.rearrange("p (k c) -> p k c", k=nk),
                    src_d[k0:k0 + nk].rearrange("k p c -> p k c"))

            def emit_dma_quarter(dst_sb, src_d, quarter):
                emit_dma_half(dst_sb, src_d, quarter, nk=NDC // 4)

            def emit_x_dma(x_t, src_d, tci, half=None, quarter=None,
                           eng=None):
                eng = eng or nc.sync
                if half is None and quarter is None:
                    eng.dma_start(
                        x_t[:].rearrange("p (k c) -> p k c", k=NDC),
                        src_d[tci].rearrange("k p c -> p k c"))
                else:
                    nk = NDC // 2 if quarter is None else NDC // 4
                    k0 = (half if quarter is None else quarter) * nk
                    eng.dma_start(
                        x_t[:, k0 * 512:(k0 + nk) * 512].rearrange(
                            "p (k c) -> p k c", k=nk),
                        src_d[tci, k0:k0 + nk].rearrange("k p c -> p k c"))

            # ---------------- projection slice generators ----------------
            # Each generator yields small emit-units (~0.4-0.9us of PE work)
            # so they can be woven between attention score-groups.
            def gen_qproj(tci, x_t, step=1):
                PHASE_LOG.append((f"qproj{tci}", nc.get_next_instruction_name()))
                psq = [pjps.tile([128, 512], F32, tag="pj",
                                 name=f"psq{tci}_{hb}") for hb in range(2)]
                for kc0 in range(0, NDC, step):
                    def unit(kc0=kc0):
                        for kc in range(kc0, kc0 + step):
                            for hb in range(2):
                                nc.tensor.matmul(
                                    psq[hb][:],
                                    wq_sb[:, kc * HF + hb * 128:
                                          kc * HF + (hb + 1) * 128],
                                    x_t[:, kc * 512:(kc + 1) * 512],
                                    start=(kc == 0), stop=(kc == NDC - 1))
                    yield unit
                def cast():
                    for hb in range(2):
                        nc.vector.tensor_scalar_add(
                            qT_sb[:, hb * S + tci * 512:
                                  hb * S + (tci + 1) * 512],
                            psq[hb][:], bq_sb[:, hb:hb + 1])
                yield cast

            def gen_kproj(tci, x_t, step=1):
                PHASE_LOG.append((f"kproj{tci}", nc.get_next_instruction_name()))
                psk = [pjps.tile([128, 512], F32, tag="pj",
                                 name=f"psk{tci}_{hb}") for hb in range(2)]
                for kc0 in range(0, NDC, step):
                    def unit(kc0=kc0):
                        for kc in range(kc0, kc0 + step):
                            for hb in range(2):
                                nc.tensor.matmul(
                                    psk[hb][:],
                                    wk_sb[:, kc * HF + hb * 128:
                                          kc * HF + (hb + 1) * 128],
                                    x_t[:, kc * 512:(kc + 1) * 512],
                                    start=(kc == 0), stop=(kc == NDC - 1))
                    yield unit
                def cast():
                    for hb in range(2):
                        nc.vector.tensor_scalar_add(
                            kT_sb[:, hb * S + tci * 512:
                                  hb * S + (tci + 1) * 512],
                            psk[hb][:], bk_sb[:, hb:hb + 1])
                yield cast

            def gen_vproj(tci, x_t, split=2):
                for ts in range(4):
                    psv = pjps.tile([128, 512], F32, tag="pj",
                                    name=f"psv{tci}_{ts}")
                    for piece in range(split):
                        def unit(ts=ts, piece=piece, psv=psv):
                            kcs_ = range(piece * NDC // split,
                                         (piece + 1) * NDC // split)
                            for kc in kcs_:
                                nc.tensor.matmul(
                                    psv[:, :HF],
                                    x_t[:, kc * 512 + ts * 128:
                                        kc * 512 + (ts + 1) * 128],
                                    wkv_sb[:, kc * HF:(kc + 1) * HF],
                                    start=(kc == 0),
                                    stop=(kc == NDC - 1 and not has_vbias))
                            if piece != split - 1:
                                return
                            if has_vbias:
                                nc.tensor.matmul(psv[:, :HF], ones_sb[:],
                                                 bkv_bf[:], start=False,
                                                 stop=True)
                            kci = tci * 4 + ts
                            dst = vaug_sb[:, kci * VROW:(kci + 1) * VROW]
                            nc.vector.tensor_copy(
                                dst.rearrange("p (h c) -> p h c", c=VW)
                                   [:, :, 0:HD],
                                psv[:, :HF].rearrange("p (h c) -> p h c",
                                                      c=HD))
                        yield unit

            def gen_vproj0_paired(x_t):
                # startup-only: v accumulates in kc-half pieces so it can
                # start as soon as the first xk/wkv halves land; two ts share
                # one atps bank (idle before attention) via the one-start
                # pending-zero trick, keeping pjps free for kproj.
                for piece in range(2):
                    for tsp in range(2):
                        def unit(piece=piece, tsp=tsp):
                            psv = vps0[tsp]
                            for ts in (2 * tsp, 2 * tsp + 1):
                                col = (ts % 2) * HF
                                for kc in range(piece * NDC // 2,
                                                (piece + 1) * NDC // 2):
                                    nc.tensor.matmul(
                                        psv[:, col:col + HF],
                                        x_t[:, kc * 512 + ts * 128:
                                            kc * 512 + (ts + 1) * 128],
                                        wkv_sb[:, kc * HF:(kc + 1) * HF],
                                        start=(ts % 2 == 0 and kc == 0),
                                        stop=(kc == NDC - 1
                                              and not has_vbias),
                                        skip_group_check=True)
                                if piece != 1:
                                    continue
                                if has_vbias:
                                    nc.tensor.matmul(
                                        psv[:, col:col + HF], ones_sb[:],
                                        bkv_bf[:], start=False, stop=True,
                                        skip_group_check=True)
                                kci = ts
                                dst = vaug_sb[:, kci * VROW:(kci + 1) * VROW]
                                nc.vector.tensor_copy(
                                    dst.rearrange("p (h c) -> p h c", c=VW)
                                       [:, :, 0:HD],
                                    psv[:, col:col + HF].rearrange(
                                        "p (h c) -> p h c", c=HD))
                        yield unit

            def gen_oproj(tok0, tail=False):
                PHASE_LOG.append((f"oproj{tok0}", nc.get_next_instruction_name()))
                obuf = obpool.tile([128, D], BF16, tag="ob",
                                   name=f"ob{tok0}")
                wide = [None]
                for of in range(2):
                    def unit(of=of):
                        ci = (tok0 // 128) % 4
                        if tail and ci == 2:
                            # post-attention all psum pools are free: rotate
                            # tail chunks over pjps/atps/stps so no chunk
                            # waits on another's psum->sbuf copy
                            if wide[0] is None:
                                wide[0] = stps.tile(
                                    [128, 512 * exp_group], F32, tag="st",
                                    name=f"pow{tok0}")
                            ops = wide[0][:, of * 512:(of + 1) * 512]
                        elif tail and ci == 1:
                            ops = atps.tile([128, 512], F32, tag="at",
                                            name=f"po{tok0}_{of}")
                        else:
                            ops = pjps.tile([128, 512], F32, tag="pj",
                                            name=f"po{tok0}_{of}")
                        for hb2 in range(2):
                            nc.tensor.matmul(
                                ops[:],
                                attnT_sb[:, hb2 * S + tok0:
                                         hb2 * S + tok0 + 128],
                                wo_sb[:, hb2 * D + of * 512:
                                      hb2 * D + (of + 1) * 512],
                                start=(hb2 == 0), stop=(hb2 == 1))
                        if tail and (of + tok0 // 128) % 2 == 0:
                            # tail: split the psum->sbuf copies across ACT
                            # and DVE (alternating per chunk) so neither
                            # engine's in-order queue paces the drain
                            nc.scalar.activation(
                                obuf[:, of * 512:(of + 1) * 512], ops[:],
                                AF.Identity, bias=0.0, scale=1.0)
                        else:
                            nc.vector.tensor_copy(
                                obuf[:, of * 512:(of + 1) * 512], ops[:])
                        if tail and tok0 == S - 128:
                            # last chunks: per-half DMAs shorten the final
                            # transfer on the critical path
                            nc.sync.dma_start(
                                out_d[tok0:tok0 + 128,
                                      of * 512:(of + 1) * 512],
                                obuf[:, of * 512:(of + 1) * 512])
                        elif of == 1:
                            nc.sync.dma_start(out_d[tok0:tok0 + 128, :],
                                              obuf[:])
                    yield unit

            def gen_load_wo():
                def unit():
                    nc.sync.dma_start(
                        wo_sb[:].rearrange("p (b c) -> p b c", b=2),
                        wo_d.rearrange("b p c -> p b c"))
                    nc.sync.dma_start(id_sb[:], id_d)
                yield unit

            def gen_load_masks():
                def unit():
                    for i in range(nmt):
                        nc.sync.dma_start(mt_sb[:, i * 128:(i + 1) * 128],
                                          mt_d[i])
                yield unit

            # ---------------- attention ----------------
            # Cross-group/head/qb deferral state: "pv" is the previous
            # group's PV emission, "norm" the completed heads' normalize+
            # transpose emissions.  Flushed after the next group's exp so
            # the in-order PE queue never head-blocks on a fresh exp.
            pend = {"pv": None, "norm": []}

            def flush_pending():
                if pend["pv"] is not None:
                    pend["pv"]()
                    pend["pv"] = None
                while pend["norm"]:
                    pend["norm"].pop(0)()

            def emit_attention(qb, extras):
                PHASE_LOG.append((f"att{qb}", nc.get_next_instruction_name()))
                q0 = qb * 512
                kcs = [kc for kc in range(NKC) if plan[qb][kc] is not None]
                # Group chunks in pairs whose exp span has no unwritten gap:
                # a pair (A, B) spans [c0_A, 512) u [512 + c0_B, ...), which
                # is contiguous iff c0_B == 0.  Pair trimmed (c0>0) chunks
                # with zero-c0 chunks (late ones, to respect weave deps);
                # leftover trimmed chunks become singleton groups.
                gaps_l = [kc for kc in kcs if plan[qb][kc][1] > 0]
                zeros_l = [kc for kc in kcs if plan[qb][kc][1] == 0]
                n_m = min(len(gaps_l), len(zeros_l))
                mixed = [[gaps_l[len(gaps_l) - n_m + i],
                          zeros_l[len(zeros_l) - n_m + i]]
                         for i in range(n_m)]
                rest_z = zeros_l[:len(zeros_l) - n_m]
                rest_g = gaps_l[:len(gaps_l) - n_m]
                groups = [rest_z[i:i + exp_group]
                          for i in range(0, len(rest_z), exp_group)]
                groups += mixed
                groups += [[kc] for kc in rest_g]
                # first valid kc per qsub + last EMITTED kc per qsub (for
                # PV psum stop flags), following the group emission order
                order = [kc for grp in groups for kc in grp]
                first_kc = {}
                last_kc = {}
                for kc in order:
                    subs = plan[qb][kc][0]
                    for j in range(4):
                        if subs[j][0] != 'Z':
                            first_kc.setdefault(j, kc)
                            last_kc[j] = kc
                front, spread = extras
                n_groups = len(groups) * HPC
                n_spread = len(spread)
                g_idx = 0
                popped = 0
                spread_start = [None]
                for h in range(HPC):
                    hb, hr = h // 2, (h % 2) * 64
                    # Bank-sized (2KB) tile: matmul start=True zeroes the
                    # whole 2KB psum bank, so only the FIRST PV matmul of
                    # this (head, qb) may carry start=True; later qsubs'
                    # first writes land on still-pending bytes and
                    # initialize correctly.
                    attn_ps = atps.tile([128, 512], F32, tag="at",
                                        name=f"at{qb}_{h}")
                    state = {"started": False}
                    for grp in groups:
                        # front extras carry dependencies for upcoming phases:
                        # pop one per group ASAP.  spread extras (oproj) are
                        # dependency-free; schedule them uniformly over the
                        # remaining groups so ACT-paced stretches keep the PE
                        # fed to the very end of the phase.
                        g_idx += 1
                        if front:
                            front.pop(0)()
                        elif spread:
                            if spread_start[0] is None:
                                spread_start[0] = g_idx
                            done = g_idx - spread_start[0] + 1
                            span = max(n_groups - spread_start[0] + 1, 1)
                            target = done * n_spread // span
                            while spread and popped < target:
                                spread.pop(0)()
                                popped += 1
                        g = len(grp)
                        st = stps.tile([128, 512 * exp_group], F32, tag="st")
                        pt = ptpool.tile([128, 512 * exp_group], BF16,
                                         tag="pt")
                        runs = []
                        for i, kc in enumerate(grp):
                            c0, c1 = plan[qb][kc][1], plan[qb][kc][2]
                            nc.tensor.matmul(
                                st[:, i * 512 + c0:i * 512 + c1],
                                kT_sb[hr:hr + 64,
                                      hb * S + kc * 128:
                                      hb * S + (kc + 1) * 128],
                                qT_sb[hr:hr + 64,
                                      hb * S + q0 + c0:hb * S + q0 + c1],
                                start=True, stop=True)
                            lo, hi = i * 512 + c0, i * 512 + c1
                            if runs and runs[-1][1] == lo:
                                runs[-1][1] = hi
                            else:
                                runs.append([lo, hi])
                        # exp only over written psum (one op per run)
                        for e0, e1 in runs:
                            nc.scalar.activation(pt[:, e0:e1], st[:, e0:e1],
                                                 AF.Exp, bias=0.0,
                                                 scale=float(SCALE))
                        for i, kc in enumerate(grp):
                            subs = plan[qb][kc][0]
                            base = i * 512
                            for j, sub in enumerate(subs):
                                if sub[0] == 'M':
                                    idx = sub[1]
                                    nc.vector.tensor_mul(
                                        pt[:, base + j * 128:
                                           base + (j + 1) * 128],
                                        pt[:, base + j * 128:
                                           base + (j + 1) * 128],
                                        mt_sb[:, idx * 128:(idx + 1) * 128])
                        # PV for this group is deferred one group so the PE
                        # queue head never waits on this group's exp.
                        flush_pending()
                        def pv(qb=qb, h=h, grp=grp, pt=pt,
                               attn_ps=attn_ps, state=state,
                               last_kc=last_kc):
                            for i, kc in enumerate(grp):
                                subs = plan[qb][kc][0]
                                base = i * 512
                                for j, sub in enumerate(subs):
                                    if sub[0] == 'Z':
                                        continue
                                    nc.tensor.matmul(
                                        attn_ps[:, j * VW:(j + 1) * VW],
                                        pt[:, base + j * 128:
                                           base + (j + 1) * 128],
                                        vaug_sb[:, kc * VROW + h * VW:
                                                kc * VROW + (h + 1) * VW],
                                        start=not state["started"],
                                        stop=(kc == last_kc[j]),
                                        skip_group_check=True)
                                    state["started"] = True
                        pend["pv"] = pv

                    def norm(qb=qb, h=h, hb=hb, q0=q0, attn_ps=attn_ps,
                             first_kc=first_kc, attn_qb=attn_qb):
                        rcp = rcpool.tile([128, 4], F32, tag="rc",
                                          name=f"rc{qb}_{h}")
                        nc.vector.reciprocal(
                            rcp[:],
                            attn_ps[:, :HPC * VW]
                            .rearrange("p (j c) -> p j c", c=VW)
                            [:, :, HD:HD + 1])
                        if all(j in first_kc for j in range(4)):
                            # all qsubs valid: one broadcast mul for the head
                            nc.vector.tensor_mul(
                                attn_qb[:, hb * 512:(hb + 1) * 512]
                                .rearrange("p (j two c) -> p j two c",
                                           j=4, two=2, c=64)[:, :, h % 2, :],
                                attn_ps[:, :HPC * VW]
                                .rearrange("p (j c) -> p j c", c=VW)
                                [:, :, 0:HD],
                                rcp[:].rearrange("p (j c) -> p j c", c=1)
                                .broadcast_to([128, 4, HD]))
                        else:
                            for j in range(4):
                                if j not in first_kc:
                                    continue
                                nc.vector.tensor_scalar_mul(
                                    attn_qb[:, hb * 512 + j * 128
                                            + (h % 2) * 64:
                                            hb * 512 + j * 128
                                            + (h % 2) * 64 + 64],
                                    attn_ps[:, j * VW:j * VW + HD],
                                    rcp[:, j:j + 1])
                        if h % 2 == 1:  # hb complete -> transpose
                            if qb == NQB - 1 and hb == 1:
                                # final block: PE transpose (scores are done,
                                # st psum is free) avoids the ~2us DMA
                                # round-trip on the tail critical path
                                tps = stps.tile([128, 512 * exp_group], BF16,
                                                tag="st", name="tps")
                                for j in range(4):
                                    nc.tensor.transpose(
                                        tps[:, j * 128:(j + 1) * 128],
                                        attn_qb[:, hb * 512 + j * 128:
                                                hb * 512 + (j + 1) * 128],
                                        id_sb[:])
                                nc.vector.tensor_copy(
                                    attnT_sb[:, hb * S + q0:
                                             hb * S + q0 + 512],
                                    tps[:, 0:512])
                            else:
                                for j in range(4):
                                    nc.sync.dma_start_transpose(
                                        attnT_sb[:, hb * S + q0 + j * 128:
                                                 hb * S + q0 + (j + 1) * 128],
                                        attn_qb[:, hb * 512 + j * 128:
                                                hb * 512 + (j + 1) * 128])
                    pend["norm"].append(norm)

            # ---------------- emission ----------------
            x_tiles = {}
            # qproj(0): weave weight-quarter DMAs with x-quarter DMAs so the
            # first matmul can start ~2.8us in.  Biases deferred (only the
            # casts need them).
            xq0 = xpool.tile([128, NDC * 512], BF16, tag="x", name="xq0")
            x_tiles[("q", 0)] = xq0
            for quarter in range(2):
                emit_dma_quarter(wq_sb, wq_d, quarter)
                emit_x_dma(xq0, xq_d, 0, quarter=quarter, eng=nc.gpsimd)
            emit_dma_half(wq_sb, wq_d, 1)
            emit_x_dma(xq0, xq_d, 0, half=1, eng=nc.gpsimd)
            nc.sync.dma_start(bqk_sb[:], bqk_d)
            for u in gen_qproj(0, xq0, step=2):
                u()
            xk0 = xpool.tile([128, NDC * 512], BF16, tag="x", name="xk0")
            x_tiles[("k", 0)] = xk0
            for half in range(2):
                emit_dma_half(wk_sb, wk_d, half)
                emit_x_dma(xk0, xk_d, 0, half=half, eng=nc.gpsimd)
                emit_dma_half(wkv_sb, wkv_d, half)
            if has_vbias:
                nc.gpsimd.memset(ones_sb[:], 1.0)
                nc.sync.dma_start(bkv_sb[:], bkv_d)
                nc.vector.tensor_copy(bkv_bf[:], bkv_sb[:])
            vps0 = [atps.tile([128, 512], F32, tag="at", name=f"vps0_{i}")
                    for i in range(2)]
            ku = list(gen_kproj(0, xk0, step=2))
            vu = list(gen_vproj0_paired(xk0))
            for u in (ku[0], ku[1], vu[0], vu[1],
                      ku[2], ku[3], ku[4], vu[2], vu[3]):
                u()
            if nmt > 0:
                for u in gen_load_masks():
                    u()
            # prefetch x for tci 1; tci 2,3 queued later
            for tci in (1, 2, 3):
                xq = xpool.tile([128, NDC * 512], BF16, tag="x",
                                name=f"xq{tci}")
                xk = xpool.tile([128, NDC * 512], BF16, tag="x",
                                name=f"xk{tci}")
                x_tiles[("q", tci)] = xq
                x_tiles[("k", tci)] = xk
            emit_x_dma(x_tiles[("q", 1)], xq_d, 1)
            emit_x_dma(x_tiles[("k", 1)], xk_d, 1)

            attn_qb = None

            def slices(*gens):
                out = []
                for g in gens:
                    out.extend(g)
                return out

            def defer_dma(*calls):
                def unit():
                    for c in calls:
                        c()
                return [unit]

            # Extras per attention phase, sized to each phase's PE deficit
            # (ACT-paced groups leave ~400ns/group of PE idle unless filled).
            # Hard deps: qT/kT of tci must exist before att(tci) scores that
            # read them; vaug of tci before att PV reaches those k-chunks.
            ex0 = (slices(
                gen_qproj(1, x_tiles[("q", 1)], step=2),
                gen_kproj(1, x_tiles[("k", 1)], step=2),
                defer_dma(lambda: emit_x_dma(x_tiles[("q", 2)], xq_d, 2),
                          lambda: emit_x_dma(x_tiles[("k", 2)], xk_d, 2)),
                gen_vproj(1, x_tiles[("k", 1)], split=1),
                gen_load_wo(),
            ), [])
            ex1 = (slices(
                gen_qproj(2, x_tiles[("q", 2)], step=2),
                gen_kproj(2, x_tiles[("k", 2)], step=2),
                defer_dma(lambda: emit_x_dma(x_tiles[("q", 3)], xq_d, 3),
                          lambda: emit_x_dma(x_tiles[("k", 3)], xk_d, 3)),
            ), [])
            ex2 = (slices(
                gen_vproj(2, x_tiles[("k", 2)], split=1),
                gen_qproj(3, x_tiles[("q", 3)], step=2),
            ), slices(
                gen_oproj(0 * 512 + 0 * 128),
                gen_oproj(0 * 512 + 1 * 128),
            ))
            ex3 = (slices(
                gen_kproj(3, x_tiles[("k", 3)], step=2),
                gen_vproj(3, x_tiles[("k", 3)], split=1),
            ), slices(
                gen_oproj(0 * 512 + 2 * 128),
                gen_oproj(0 * 512 + 3 * 128),
                gen_oproj(1 * 512 + 0 * 128),
                gen_oproj(1 * 512 + 1 * 128),
                gen_oproj(1 * 512 + 2 * 128),
                gen_oproj(1 * 512 + 3 * 128),
                gen_oproj(2 * 512 + 0 * 128),
                gen_oproj(2 * 512 + 1 * 128),
                gen_oproj(2 * 512 + 2 * 128),
                gen_oproj(2 * 512 + 3 * 128),
            ))
            extras = [ex0, ex1, ex2, ex3]
            for qb in range(NQB):
                attn_qb = anpool.tile([128, 1024], BF16, tag="an",
                                      name=f"an{qb}")
                emit_attention(qb, extras[qb])
                for lst in extras[qb]:
                    while lst:
                        lst.pop(0)()
            flush_pending()
            for t in range(4):
                for u in gen_oproj(3 * 512 + t * 128, tail=True):
                    u()
    nc.compile()
    return nc


_CACHE = {}
PHASE_LOG = []
VARIANT = {"exp_group": 2}


def _get_nc(plan, nmt, has_vbias):
    key = (repr(plan), nmt, has_vbias, repr(sorted(VARIANT.items())))
    if key not in _CACHE:
        _CACHE[key] = _build_nc(plan, nmt, has_vbias, **VARIANT)
    return _CACHE[key]


def shard_inputs(queries, keys, mask, Wq, bq, Wk, bk, Wv, bv, Wo, bo):
    """Host-side prep: returns (in_maps, plan, nmt, has_vbias)."""
    Wkv = (Wk.astype(np.float64) @ Wv.astype(np.float64)).astype(np.float32)
    bkv = (bk.astype(np.float64) @ Wv.astype(np.float64)
           + bv.astype(np.float64)).astype(np.float32)
    has_vbias = bool(np.any(bkv != 0.0))

    plan, tiles_list = _classify_mask(np.asarray(mask))
    nmt = len(tiles_list)
    assert nmt <= 64, f"too many distinct mask tiles ({nmt})"
    if nmt > 0:
        mtiles = np.stack(tiles_list).astype(ml_dtypes.bfloat16)
    else:
        mtiles = np.zeros((1, 128, 128), dtype=ml_dtypes.bfloat16)

    def pack_x(x):
        # [S, D] -> [tci, kc, 128 d, 512 tok] bf16
        return np.ascontiguousarray(
            x.reshape(NQB, 512, NDC, 128).transpose(0, 2, 3, 1)
        ).astype(ml_dtypes.bfloat16)

    def pack_w(w, cols):
        return np.ascontiguousarray(
            w[:, cols].reshape(NDC, 128, HF)).astype(ml_dtypes.bfloat16)

    in_maps = []
    for c in range(NCORES):
        b, g = c // 4, c % 4
        cols = slice(HF * g, HF * (g + 1))
        in_maps.append({
            "xq": pack_x(queries[b]),
            "xk": pack_x(keys[b]),
            "wq": pack_w(Wq, cols),
            "wk": pack_w(Wk, cols),
            "wkv": pack_w(Wkv, cols),
            "wo": np.ascontiguousarray(
                Wo[cols, :].reshape(2, 128, D)).astype(ml_dtypes.bfloat16),
            "bqk": np.ascontiguousarray(np.concatenate(
                [bq[cols].reshape(2, 128).T, bk[cols].reshape(2, 128).T],
                axis=1)),
            "bkv": bkv[cols].reshape(1, HF).copy(),
            "mtiles": mtiles,
            "ident": np.eye(128, dtype=ml_dtypes.bfloat16),
        })
    return in_maps, plan, nmt, has_vbias


def combine_outputs(results, bo):
    out = np.empty((B, S, D), dtype=np.float32)
    for b in range(B):
        acc = results[4 * b]["out"].astype(np.float32)
        for g in range(1, 4):
            acc = acc + results[4 * b + g]["out"].astype(np.float32)
        out[b] = acc + bo[None, :]
    return out


def kernel(queries, keys, values, mask, Wq, bq, Wk, bk, Wv, bv, Wo, bo,
           _trace=False, _result_holder=None):
    queries = np.asarray(queries, dtype=np.float32)
    keys = np.asarray(keys, dtype=np.float32)
    mask = np.asarray(mask)
    in_maps, plan, nmt, has_vbias = shard_inputs(
        queries, keys, mask,
        np.asarray(Wq, np.float32), np.asarray(bq, np.float32),
        np.asarray(Wk, np.float32), np.asarray(bk, np.float32),
        np.asarray(Wv, np.float32), np.asarray(bv, np.float32),
        np.asarray(Wo, np.float32), np.asarray(bo, np.float32))
    nc = _get_nc(plan, nmt, has_vbias)
    res = run_bass_kernel_spmd(nc, in_maps, core_ids=list(range(NCORES)),
                               trace=_trace)
    if _result_holder is not None:
        _result_holder.append(res)
    return combine_outputs(res.results, np.asarray(bo, np.float32))
